# revision 12
# baseline (speedup 1.0000x reference)
"""DeepFM (embedding gather + FM + 5-layer seq-1 attention + head) on 8 trn2 cores.

Strategy: data-parallel over batch (2048 rows/core). Inside each core:
  - fused emb1+emb2 table (F*V, 17) f32; per-(tile,field) indirect-DMA gathers
    (128 rows x 17 f32 per op) into batch-major G tiles
  - Xv scaling, L2-norm over fields, FM second-order stats via strided DVE/ACT
  - PE transposes to feature-major; attention stack as float32r (e8m11) matmuls
    at full PE rate, weights/biases host-packed into lhsT layouts
  - all linear biases folded in via augmented ones-rows / bias rows
"""
import numpy as np

import concourse.bass as bass
import concourse.mybir as mybir
from concourse.tile import TileContext
from concourse.vector_clock import ScopedClock
from concourse.masks import make_identity
from concourse.bass_utils import run_bass_kernel_spmd

F32 = mybir.dt.float32
F32R = mybir.dt.float32r
I32 = mybir.dt.int32
AF = mybir.ActivationFunctionType
OP = mybir.AluOpType

# problem constants (hardcoded per contract)
N = 16384
F = 39
V = 100000
E = 16
EW = E + 1          # fused [emb1 | emb2] row width
D = F * E           # 624
L = 5
NCORES = 8
NPC = N // NCORES   # 2048 rows per core
NT = NPC // 128     # 16 tiles of 128 rows
NB = 256            # matmul batch-chunk (free dim); >=256 required for f32r rate
NCH = NPC // NB     # 8 chunks
KDIMS = [128, 128, 128, 128, 112]   # K-tiles over 624
KDIMS_AUG = [128, 128, 128, 128, 113]  # incl. ones/bias row at 112 of tail tile
MDIMS = [128, 128, 128, 128, 112]

MAX_WAITS = 1

LAST_RESULT = None  # test harness reads exec_time_ns from here


class SplitWaitTileContext(TileContext):
    """Walrus (CoreV3) accepts at most one sync-wait command per instruction;
    Tile can emit several. Split extras onto preceding same-engine NOPs, and
    do the same for the kernel-tail drain."""

    def _add_instruction(self, inst):
        si = inst.sync_info
        if si is not None and len(si.on_wait) > MAX_WAITS:
            waits = list(si.on_wait)
            head, tail = waits[:-MAX_WAITS], waits[-MAX_WAITS:]
            for i in range(0, len(head), MAX_WAITS):
                nop = mybir.InstNoOp(
                    name=self.nc.get_next_instruction_name(),
                    sync_info=mybir.SyncInfo(
                        on_wait=head[i : i + MAX_WAITS], on_update=[]
                    ),
                    bass_nofuse=True,
                    engine=inst.engine,
                )
                super()._add_instruction(nop)
            inst.sync_info = mybir.SyncInfo(on_wait=tail, on_update=si.on_update)
        super()._add_instruction(inst)

    def _drain_and_barrier(self, tick_clock, wait_clock):
        nc = self.nc
        probe = nc.sync.nop(nofuse=True, hint="tail_wait_probe")
        wait_clock.add_sem_waits(
            probe.ins, ScopedClock({None: tick_clock.global_clock})
        )
        waits = list(probe.ins.sync_info.on_wait)
        probe.ins.sync_info.on_wait = waits[:MAX_WAITS]
        for i in range(MAX_WAITS, len(waits), MAX_WAITS):
            nop = nc.sync.nop(nofuse=True, hint="tail_wait_split")
            nop.ins.sync_info = mybir.SyncInfo(
                on_wait=waits[i : i + MAX_WAITS], on_update=[]
            )
        drain_inst = nc.sync.drain()
        wait_clock.add_sem_waits(
            drain_inst.ins, ScopedClock({None: tick_clock.global_clock})
        )
        if len(drain_inst.ins.sync_info.on_wait) > MAX_WAITS:
            drain_inst.ins.sync_info.on_wait = []
        nc.all_engine_barrier()
        assert self.sems is not None
        popped = nc._tile_sem_poison_stack.pop()
        assert popped is self._sem_poison
        nc.clear_and_free_semaphores(list(self.sems.allocated().values()))
        nc.all_engine_barrier()


def round_f32r(a):
    """Round-to-nearest-even fp32 -> e8m11 (low 12 mantissa bits zero)."""
    a = np.ascontiguousarray(a, dtype=np.float32)
    u = a.view(np.uint32)
    r = (u + np.uint32(0x7FF) + ((u >> np.uint32(12)) & np.uint32(1))) & np.uint32(
        0xFFFFF000
    )
    return r.view(np.float32)


def build_nc():
    nc = bass.Bass()

    tab = nc.declare_dram_parameter("tab", [F * V, EW], F32, isOutput=False)
    idx = nc.declare_dram_parameter("idx", [128, NT * F], I32, isOutput=False)
    xv = nc.declare_dram_parameter("xv", [128, NT * F], F32, isOutput=False)
    wq = nc.declare_dram_parameter("wq", [L, 128, 5 * D], F32R, isOutput=False)
    wk = nc.declare_dram_parameter("wk", [L, 128, 5 * D], F32R, isOutput=False)
    wv = nc.declare_dram_parameter("wv", [L, 128, 5 * D], F32R, isOutput=False)
    wo = nc.declare_dram_parameter("wo", [L, 128, 5 * D], F32R, isOutput=False)
    am = nc.declare_dram_parameter("am", [128, 5 * F], F32R, isOutput=False)
    bm = nc.declare_dram_parameter("bm", [F, 5 * 128], F32R, isOutput=False)
    m12 = nc.declare_dram_parameter("m12", [56, 9], F32R, isOutput=False)
    m3 = nc.declare_dram_parameter("m3", [128, 5 * 4], F32R, isOutput=False)
    ffw9 = nc.declare_dram_parameter("ffw9", [9, 13], F32R, isOutput=False)
    ffw3 = nc.declare_dram_parameter("ffw3", [4, 13], F32R, isOutput=False)
    fdw = nc.declare_dram_parameter("fdw", [13, 2], F32R, isOutput=False)
    bo_d = nc.declare_dram_parameter("bo_d", [1, L * 640], F32R, isOutput=False)
    onesw = nc.declare_dram_parameter("onesw", [1, NPC], F32R, isOutput=False)
    y = nc.declare_dram_parameter("y", [NPC, 2], F32, isOutput=True)

    with SplitWaitTileContext(nc) as tc:
        with (
            tc.tile_pool(name="const", bufs=1) as cp,
            tc.tile_pool(name="persist", bufs=1) as pp,
            tc.tile_pool(name="wpool", bufs=1) as wp,
            tc.tile_pool(name="work", bufs=2) as wk_p,
            tc.tile_pool(name="gpool", bufs=2) as gp,
            tc.tile_pool(name="small", bufs=2) as sp,
            tc.tile_pool(name="psA", bufs=2, space="PSUM") as ps_qk,
            tc.tile_pool(name="psB", bufs=1, space="PSUM") as ps_s,
            tc.tile_pool(name="psC", bufs=2, space="PSUM") as ps_vb,
            tc.tile_pool(name="psD", bufs=2, space="PSUM") as ps_o,
            tc.tile_pool(name="psT", bufs=1, space="PSUM") as ps_t,
        ):
            # ---- constants / persistent tiles ----
            idx_sb = cp.tile([128, NT * F], I32)
            nc.sync.dma_start(out=idx_sb[:], in_=idx[:])
            xv_sb = cp.tile([128, NT * F], F32)
            nc.sync.dma_start(out=xv_sb[:], in_=xv[:])
            a_sb = cp.tile([128, 5 * F], F32R)
            nc.sync.dma_start(out=a_sb[:], in_=am[:])
            b_sb = cp.tile([F, 5 * 128], F32R)
            nc.sync.dma_start(out=b_sb[:], in_=bm[:])
            m12_sb = cp.tile([56, 9], F32R)
            nc.sync.dma_start(out=m12_sb[:], in_=m12[:])
            m3_sb = cp.tile([128, 5 * 4], F32R)
            nc.sync.dma_start(out=m3_sb[:], in_=m3[:])
            ffw9_sb = cp.tile([9, 13], F32R)
            nc.sync.dma_start(out=ffw9_sb[:], in_=ffw9[:])
            ffw3_sb = cp.tile([4, 13], F32R)
            nc.sync.dma_start(out=ffw3_sb[:], in_=ffw3[:])
            fdw_sb = cp.tile([13, 2], F32R)
            nc.sync.dma_start(out=fdw_sb[:], in_=fdw[:])
            ident = cp.tile([128, 128], F32)
            make_identity(nc, ident[:])
            bo_sb = cp.tile([1, L * 640], F32R)
            nc.sync.dma_start(out=bo_sb[:], in_=bo_d[:])
            ones_t = cp.tile([1, NB], F32R)
            nc.sync.dma_start(out=ones_t[:], in_=onesw[:, :NB])

            XVT = [pp.tile([128, NPC], F32R, tag=f"xvt{i}", name=f"xvt{i}") for i in range(5)]
            OT = [pp.tile([128, NPC], F32R, tag=f"ot{i}", name=f"ot{i}") for i in range(5)]
            headT = pp.tile([56, NPC], F32R, tag="headT")
            out_sb = pp.tile([128, NT * 2], F32, tag="outsb")

            # ---- phase 0: gather + FM + normalize + transpose, per 128-row tile
            for t in range(NT):
                g = gp.tile([128, F * EW], F32, tag="g")
                for f in range(F):
                    col = t * F + f
                    nc.gpsimd.indirect_dma_start(
                        out=g[:, f * EW : (f + 1) * EW],
                        out_offset=None,
                        in_=tab[:],
                        in_offset=bass.IndirectOffsetOnAxis(
                            ap=idx_sb[:, col : col + 1], axis=0
                        ),
                    )
                g3 = g[:].rearrange("p (f j) -> p f j", j=EW)
                # scale by Xv (also scales the emb1 slot -> f1w)
                nc.vector.tensor_tensor(
                    out=g3,
                    in0=g3,
                    in1=xv_sb[:, t * F : (t + 1) * F]
                    .unsqueeze(2)
                    .to_broadcast([128, F, EW]),
                    op=OP.mult,
                )
                g_xv = g3[:, :, 1:]                      # (p, f, e)
                g_ef = g_xv.transpose([0, 2, 1])         # (p, e, f) view
                xvt_tmp = wk_p.tile([128, D + 1], F32, tag="xvtmp")
                nc.vector.memset(xvt_tmp[:, D : D + 1], 1.0)
                sq_v = xvt_tmp[:, :D].rearrange("p (e f) -> p e f", f=F)
                nc.scalar.activation(out=sq_v, in_=g_ef, func=AF.Square)
                ss = sp.tile([128, 16], F32, tag="ss")
                nc.vector.reduce_sum(out=ss[:], in_=sq_v, axis=mybir.AxisListType.X)
                tt = sp.tile([128, 16], F32, tag="tt")
                nc.vector.reduce_sum(out=tt[:], in_=g_ef, axis=mybir.AxisListType.X)
                mx = sp.tile([128, 16], F32, tag="mx")
                nc.vector.tensor_scalar_max(out=mx[:], in0=ss[:], scalar1=1e-24)
                rt = sp.tile([128, 16], F32, tag="rt")
                nc.scalar.sqrt(out=rt[:], in_=mx[:])
                inv = sp.tile([128, 16], F32, tag="inv")
                nc.vector.reciprocal(out=inv[:], in_=rt[:])
                # normalized xv, contiguous (f, e) layout — reuses sq storage
                xv_v = xvt_tmp[:, :D].rearrange("p (f e) -> p f e", e=E)
                nc.vector.tensor_tensor(
                    out=xv_v,
                    in0=g_xv,
                    in1=inv[:].unsqueeze(1).to_broadcast([128, F, E]),
                    op=OP.mult,
                )
                # head tile: [f1w | f2 | 1]
                head = wk_p.tile([128, 56], F32, tag="head")
                nc.vector.memset(head[:, 55:56], 1.0)
                nc.scalar.activation(out=head[:, 0:F], in_=g3[:, :, 0], func=AF.Copy)
                u = sp.tile([128, 16], F32, tag="u")
                nc.vector.tensor_tensor(out=u[:], in0=tt[:], in1=inv[:], op=OP.mult)
                u2 = sp.tile([128, 16], F32, tag="u2")
                nc.vector.tensor_tensor(out=u2[:], in0=u[:], in1=u[:], op=OP.mult)
                w1 = sp.tile([128, 16], F32, tag="w1")
                nc.vector.tensor_tensor(out=w1[:], in0=ss[:], in1=inv[:], op=OP.mult)
                w2 = sp.tile([128, 16], F32, tag="w2")
                nc.vector.tensor_tensor(out=w2[:], in0=w1[:], in1=inv[:], op=OP.mult)
                dd = sp.tile([128, 16], F32, tag="dd")
                nc.vector.tensor_tensor(
                    out=dd[:], in0=u2[:], in1=w2[:], op=OP.subtract
                )
                nc.vector.tensor_scalar_mul(out=head[:, F:55], in0=dd[:], scalar1=0.5)
                # transposes to feature-major
                for cc in range(5):
                    w = 128 if cc < 4 else 113   # tail chunk carries the ones col
                    pt = ps_t.tile([128, 128], F32, tag="pt")
                    nc.tensor.transpose(
                        out=pt[:w, :],
                        in_=xvt_tmp[:, cc * 128 : cc * 128 + w],
                        identity=ident[:],
                    )
                    nc.vector.tensor_copy(
                        out=XVT[cc][:w, t * 128 : (t + 1) * 128], in_=pt[:w, :]
                    )
                pt = ps_t.tile([128, 128], F32, tag="pt")
                nc.tensor.transpose(out=pt[:56, :], in_=head[:], identity=ident[:])
                nc.vector.tensor_copy(
                    out=headT[:, t * 128 : (t + 1) * 128], in_=pt[:56, :]
                )

            # ---- attention layers ----
            for l in range(L):
                wq_t = wp.tile([128, 5 * D], F32R, tag="wq")
                nc.sync.dma_start(out=wq_t[:], in_=wq[l, :, :])
                wk_t = wp.tile([128, 5 * D], F32R, tag="wk")
                nc.sync.dma_start(out=wk_t[:], in_=wk[l, :, :])
                wv_t = wp.tile([128, 5 * D], F32R, tag="wv")
                nc.sync.dma_start(out=wv_t[:], in_=wv[l, :, :])
                wo_t = wp.tile([128, 5 * D], F32R, tag="wo")
                nc.sync.dma_start(out=wo_t[:], in_=wo[l, :, :])
                Xsrc = XVT if l == 0 else OT
                for c in range(NCH):
                    cs = slice(c * NB, (c + 1) * NB)
                    # scores
                    pss = ps_s.tile([F, NB], F32, tag="s")
                    for m in range(5):
                        mw = MDIMS[m]
                        psq = ps_qk.tile([128, NB], F32, tag="qk")
                        for kb in range(5):
                            kw = KDIMS_AUG[kb]
                            nc.tensor.matmul(
                                out=psq[:mw, :],
                                lhsT=wq_t[:kw, kb * D + m * 128 : kb * D + m * 128 + mw],
                                rhs=XVT[kb][:kw, cs],
                                start=(kb == 0),
                                stop=(kb == 4),
                            )
                        q_sb = wk_p.tile([128, NB], F32, tag="qsb")
                        nc.scalar.activation(
                            out=q_sb[:mw, :], in_=psq[:mw, :], func=AF.Copy
                        )
                        psk = ps_qk.tile([128, NB], F32, tag="qk")
                        for kb in range(5):
                            kw = KDIMS_AUG[kb]
                            nc.tensor.matmul(
                                out=psk[:mw, :],
                                lhsT=wk_t[:kw, kb * D + m * 128 : kb * D + m * 128 + mw],
                                rhs=XVT[kb][:kw, cs],
                                start=(kb == 0),
                                stop=(kb == 4),
                            )
                        p_sb = wk_p.tile([128, NB], F32R, tag="psb")
                        nc.vector.tensor_tensor(
                            out=p_sb[:mw, :],
                            in0=q_sb[:mw, :],
                            in1=psk[:mw, :],
                            op=OP.mult,
                        )
                        nc.tensor.matmul(
                            out=pss[:, :],
                            lhsT=a_sb[:mw, m * F : (m + 1) * F],
                            rhs=p_sb[:mw, :],
                            start=(m == 0),
                            stop=(m == 4),
                        )
                    s_sb = wk_p.tile([F, NB], F32R, tag="ssb")
                    nc.vector.tensor_copy(out=s_sb[:], in_=pss[:])
                    # v, broadcast, att
                    at_tiles = []
                    for m in range(5):
                        mw = MDIMS[m]
                        psv = ps_vb.tile([128, NB], F32, tag="vb")
                        for kb in range(5):
                            kw = KDIMS_AUG[kb]
                            nc.tensor.matmul(
                                out=psv[:mw, :],
                                lhsT=wv_t[:kw, kb * D + m * 128 : kb * D + m * 128 + mw],
                                rhs=Xsrc[kb][:kw, cs],
                                start=(kb == 0),
                                stop=(kb == 4),
                            )
                        v_sb = wk_p.tile([128, NB], F32, tag="vsb")
                        nc.scalar.activation(
                            out=v_sb[:mw, :], in_=psv[:mw, :], func=AF.Copy
                        )
                        psb = ps_vb.tile([128, NB], F32, tag="vb")
                        nc.tensor.matmul(
                            out=psb[:mw, :],
                            lhsT=b_sb[:, m * 128 : m * 128 + mw],
                            rhs=s_sb[:],
                            start=True,
                            stop=True,
                        )
                        at = wk_p.tile([128, NB], F32R, tag=f"at{m}")
                        nc.vector.tensor_tensor(
                            out=at[:mw, :],
                            in0=v_sb[:mw, :],
                            in1=psb[:mw, :],
                            op=OP.mult,
                        )
                        at_tiles.append(at)
                    # out projection (+bo via ones row)
                    for m in range(5):
                        mw = MDIMS[m]
                        ow = 113 if m == 4 else mw   # +1 row: ones for bias path
                        pso = ps_o.tile([128, NB], F32, tag="o")
                        for kb in range(5):
                            kw = KDIMS[kb]
                            nc.tensor.matmul(
                                out=pso[:mw, :],
                                lhsT=wo_t[:kw, kb * D + m * 128 : kb * D + m * 128 + mw],
                                rhs=at_tiles[kb][:kw, :],
                                start=(kb == 0),
                                stop=False,
                            )
                        nc.tensor.matmul(
                            out=pso[:ow, :],
                            lhsT=bo_sb[:, l * 640 + m * 128 : l * 640 + m * 128 + ow],
                            rhs=ones_t[:, :],
                            start=False,
                            stop=True,
                        )
                        nc.vector.tensor_copy(out=OT[m][:ow, cs], in_=pso[:ow, :])

            # ---- head ----
            for c in range(NCH):
                cs = slice(c * NB, (c + 1) * NB)
                ffin12 = wk_p.tile([9, NB], F32R, tag="ffin12")
                ffin3 = wk_p.tile([4, NB], F32R, tag="ffin3")
                ffout = wk_p.tile([13, NB], F32R, tag="ffout")
                ph12 = ps_t.tile([9, NB], F32, tag="pt")
                nc.tensor.matmul(
                    out=ph12[:, :],
                    lhsT=m12_sb[:],
                    rhs=headT[:, cs],
                    start=True,
                    stop=True,
                )
                nc.vector.tensor_copy(out=ffin12[:], in_=ph12[:, :])
                ph3 = ps_t.tile([4, NB], F32, tag="pt")
                for kb in range(5):
                    kw = KDIMS_AUG[kb]
                    nc.tensor.matmul(
                        out=ph3[:, :],
                        lhsT=m3_sb[:kw, kb * 4 : (kb + 1) * 4],
                        rhs=OT[kb][:kw, cs],
                        start=(kb == 0),
                        stop=(kb == 4),
                    )
                nc.vector.tensor_copy(out=ffin3[:], in_=ph3[:, :])
                pf = ps_t.tile([13, NB], F32, tag="pt")
                nc.tensor.matmul(
                    out=pf[:, :], lhsT=ffw9_sb[:], rhs=ffin12[:],
                    start=True, stop=False,
                )
                nc.tensor.matmul(
                    out=pf[:, :], lhsT=ffw3_sb[:], rhs=ffin3[:],
                    start=False, stop=True,
                )
                nc.vector.tensor_scalar_max(out=ffout[:], in0=pf[:, :], scalar1=0.0)
                for q in range(NB // 128):
                    cc = (c * NB) // 128 + q
                    ptot = ps_t.tile([128, 2], F32, tag="pt")
                    nc.tensor.matmul(
                        out=ptot[:, :],
                        lhsT=ffout[:, q * 128 : (q + 1) * 128],
                        rhs=fdw_sb[:],
                        start=True,
                        stop=True,
                    )
                    nc.vector.tensor_copy(
                        out=out_sb[:, cc * 2 : (cc + 1) * 2], in_=ptot[:, :]
                    )

            # final store: out_sb (128, NT, 2) -> y (NT*128, 2)
            nc.sync.dma_start(
                out=y[:].rearrange("(t p) j -> p t j", p=128),
                in_=out_sb[:].rearrange("p (t j) -> p t j", j=2),
            )

    return nc


def host_pack(Xi, Xv, emb1, emb2, Wq, bq, Wk, bk, Wv, bv, Wo, bo,
              m1_w, m1_b, m2_w, m2_b, m3_w, m3_b, ffw_w, ffw_b, fd_w, fd_b):
    """Preprocess full inputs into per-core input maps."""
    idxg = (
        np.arange(F, dtype=np.int64)[None, :] * V + np.asarray(Xi)[:, :, 0]
    ).astype(np.int32)                                    # (N, F)
    Xv = np.asarray(Xv, dtype=np.float32)
    tab = np.concatenate(
        [np.asarray(emb1).reshape(F * V, 1), np.asarray(emb2).reshape(F * V, E)],
        axis=1,
    ).astype(np.float32)                                  # (F*V, 17)

    def pack_w(Wx, bx):
        # (L, D, D)+(L, D) -> (L, 128, 5, D) lhsT tiles, bias at row 112 of kb=4
        out = np.zeros((L, 128, 5, D), dtype=np.float32)
        Wx = np.asarray(Wx, dtype=np.float32)
        for kb in range(5):
            kw = KDIMS[kb]
            out[:, :kw, kb, :] = Wx[:, kb * 128 : kb * 128 + kw, :]
        out[:, 112, 4, :] = np.asarray(bx, dtype=np.float32)
        return round_f32r(out).reshape(L, 128, 5 * D)

    wq_h = pack_w(Wq, bq)
    wk_h = pack_w(Wk, bk)
    wv_h = pack_w(Wv, bv)
    wo_h = pack_w(Wo, bo)

    am_h = np.zeros((128, 5, F), dtype=np.float32)
    for kb in range(5):
        for p in range(KDIMS[kb]):
            d = kb * 128 + p
            am_h[p, kb, d // 16] = 0.25
    am_h = round_f32r(am_h).reshape(128, 5 * F)

    bm_h = np.zeros((F, 5, 128), dtype=np.float32)
    for m in range(5):
        for p in range(MDIMS[m]):
            d = m * 128 + p
            bm_h[d // 16, m, p] = 1.0
    bm_h = round_f32r(bm_h).reshape(F, 5 * 128)

    m12_h = np.zeros((56, 9), dtype=np.float32)
    m12_h[:F, 0:4] = np.asarray(m1_w, dtype=np.float32)
    m12_h[F:55, 4:8] = np.asarray(m2_w, dtype=np.float32)
    m12_h[55, 0:4] = np.asarray(m1_b, dtype=np.float32)
    m12_h[55, 4:8] = np.asarray(m2_b, dtype=np.float32)
    m12_h[55, 8] = 1.0   # ones-row producer (headT row 55 is all-ones)
    m12_h = round_f32r(m12_h)

    m3_h = np.zeros((128, 5, 4), dtype=np.float32)
    for kb in range(5):
        kw = KDIMS[kb]
        m3_h[:kw, kb, :] = np.asarray(m3_w, dtype=np.float32)[
            kb * 128 : kb * 128 + kw, :
        ]
    m3_h[112, 4, :] = np.asarray(m3_b, dtype=np.float32)
    m3_h = round_f32r(m3_h).reshape(128, 5 * 4)

    ffw_w = np.asarray(ffw_w, dtype=np.float32)
    ffw9_h = np.zeros((9, 13), dtype=np.float32)
    ffw9_h[0:8, 0:12] = ffw_w[0:8]
    ffw9_h[8, 0:12] = np.asarray(ffw_b, dtype=np.float32)
    ffw9_h[8, 12] = 1.0   # ones-row producer (ffin12 row 8 is all-ones)
    ffw9_h = round_f32r(ffw9_h)
    ffw3_h = np.zeros((4, 13), dtype=np.float32)
    ffw3_h[:, 0:12] = ffw_w[8:12]
    ffw3_h = round_f32r(ffw3_h)

    fdw_h = np.zeros((13, 2), dtype=np.float32)
    fdw_h[:12] = np.asarray(fd_w, dtype=np.float32)
    fdw_h[12] = np.asarray(fd_b, dtype=np.float32)
    fdw_h = round_f32r(fdw_h)

    bo_h = np.zeros((L, 5, 128), dtype=np.float32)
    bo_a = np.asarray(bo, dtype=np.float32)
    for m in range(5):
        mw = MDIMS[m]
        bo_h[:, m, :mw] = bo_a[:, m * 128 : m * 128 + mw]
    bo_h[:, 4, 112] = 1.0   # ones-row producer for OT[4] bias path
    bo_h = round_f32r(bo_h).reshape(1, L * 640)
    ones_h = np.ones((1, NPC), dtype=np.float32)

    in_maps = []
    for core in range(NCORES):
        sl = slice(core * NPC, (core + 1) * NPC)
        idx_r = (
            idxg[sl].reshape(NT, 128, F).transpose(1, 0, 2).reshape(128, NT * F)
        )
        xv_r = Xv[sl].reshape(NT, 128, F).transpose(1, 0, 2).reshape(128, NT * F)
        in_maps.append(
            dict(
                tab=tab,
                idx=np.ascontiguousarray(idx_r),
                xv=np.ascontiguousarray(xv_r),
                wq=wq_h, wk=wk_h, wv=wv_h, wo=wo_h,
                am=am_h, bm=bm_h, m12=m12_h, m3=m3_h,
                ffw9=ffw9_h, ffw3=ffw3_h, fdw=fdw_h,
                bo_d=bo_h, onesw=ones_h,
            )
        )
    return in_maps


_NC_CACHE = None


def kernel(**inputs):
    global _NC_CACHE, LAST_RESULT
    in_maps = host_pack(**inputs)
    if _NC_CACHE is None:
        _NC_CACHE = build_nc()
    res = run_bass_kernel_spmd(_NC_CACHE, in_maps, list(range(NCORES)))
    LAST_RESULT = res
    out = np.concatenate([res.results[c]["y"] for c in range(NCORES)], axis=0)
    return out


if __name__ == "__main__":
    rng = np.random.default_rng(0)
    print("building...")
    nc = build_nc()
    print("built ok")


# revision 15
# speedup vs baseline: 1.2052x; 1.2052x over previous
"""DeepFM (embedding gather + FM + 5-layer seq-1 attention + head) on 8 trn2 cores.

Strategy: data-parallel over batch (2048 rows/core). Inside each core:
  - fused emb1+emb2 table (F*V, 17) f32; per-(tile,field) indirect-DMA gathers
    (128 rows x 17 f32 per op) into batch-major G tiles
  - Xv scaling, L2-norm over fields, FM second-order stats via strided DVE/ACT
  - PE transposes to feature-major; attention stack as float32r (e8m11) matmuls
    at full PE rate, weights/biases host-packed into lhsT layouts
  - all linear biases folded in via augmented ones-rows / bias rows
"""
import numpy as np

import concourse.bass as bass
import concourse.mybir as mybir
from concourse.tile import TileContext
from concourse.vector_clock import ScopedClock
from concourse.masks import make_identity
from concourse.bass_utils import run_bass_kernel_spmd

F32 = mybir.dt.float32
F32R = mybir.dt.float32r
I32 = mybir.dt.int32
AF = mybir.ActivationFunctionType
OP = mybir.AluOpType

# problem constants (hardcoded per contract)
N = 16384
F = 39
V = 100000
E = 16
EW = E + 1          # fused [emb1 | emb2] row width
D = F * E           # 624
L = 5
NCORES = 8
NPC = N // NCORES   # 2048 rows per core
NT = NPC // 128     # 16 tiles of 128 rows
NB = 512            # matmul batch-chunk (free dim); >=256 required for f32r rate
NCH = NPC // NB     # 8 chunks
KDIMS = [128, 128, 128, 128, 112]   # K-tiles over 624
KDIMS_AUG = [128, 128, 128, 128, 113]  # incl. ones/bias row at 112 of tail tile
MDIMS = [128, 128, 128, 128, 112]

MAX_WAITS = 1

LAST_RESULT = None  # test harness reads exec_time_ns from here


class SplitWaitTileContext(TileContext):
    """Walrus (CoreV3) accepts at most one sync-wait command per instruction;
    Tile can emit several. Split extras onto preceding same-engine NOPs, and
    do the same for the kernel-tail drain."""

    def _add_instruction(self, inst):
        si = inst.sync_info
        if si is not None and len(si.on_wait) > MAX_WAITS:
            waits = list(si.on_wait)
            head, tail = waits[:-MAX_WAITS], waits[-MAX_WAITS:]
            for i in range(0, len(head), MAX_WAITS):
                nop = mybir.InstNoOp(
                    name=self.nc.get_next_instruction_name(),
                    sync_info=mybir.SyncInfo(
                        on_wait=head[i : i + MAX_WAITS], on_update=[]
                    ),
                    bass_nofuse=True,
                    engine=inst.engine,
                )
                super()._add_instruction(nop)
            inst.sync_info = mybir.SyncInfo(on_wait=tail, on_update=si.on_update)
        super()._add_instruction(inst)

    def _drain_and_barrier(self, tick_clock, wait_clock):
        nc = self.nc
        probe = nc.sync.nop(nofuse=True, hint="tail_wait_probe")
        wait_clock.add_sem_waits(
            probe.ins, ScopedClock({None: tick_clock.global_clock})
        )
        waits = list(probe.ins.sync_info.on_wait)
        probe.ins.sync_info.on_wait = waits[:MAX_WAITS]
        for i in range(MAX_WAITS, len(waits), MAX_WAITS):
            nop = nc.sync.nop(nofuse=True, hint="tail_wait_split")
            nop.ins.sync_info = mybir.SyncInfo(
                on_wait=waits[i : i + MAX_WAITS], on_update=[]
            )
        drain_inst = nc.sync.drain()
        wait_clock.add_sem_waits(
            drain_inst.ins, ScopedClock({None: tick_clock.global_clock})
        )
        if len(drain_inst.ins.sync_info.on_wait) > MAX_WAITS:
            drain_inst.ins.sync_info.on_wait = []
        nc.all_engine_barrier()
        assert self.sems is not None
        popped = nc._tile_sem_poison_stack.pop()
        assert popped is self._sem_poison
        nc.clear_and_free_semaphores(list(self.sems.allocated().values()))
        nc.all_engine_barrier()


def round_f32r(a):
    """Round-to-nearest-even fp32 -> e8m11 (low 12 mantissa bits zero)."""
    a = np.ascontiguousarray(a, dtype=np.float32)
    u = a.view(np.uint32)
    r = (u + np.uint32(0x7FF) + ((u >> np.uint32(12)) & np.uint32(1))) & np.uint32(
        0xFFFFF000
    )
    return r.view(np.float32)


def build_nc():
    nc = bass.Bass()

    tab = nc.declare_dram_parameter("tab", [F * V, EW], F32, isOutput=False)
    idx = nc.declare_dram_parameter("idx", [128, NT * F], I32, isOutput=False)
    xv = nc.declare_dram_parameter("xv", [128, NT * F], F32, isOutput=False)
    wq = nc.declare_dram_parameter("wq", [L, 128, 5 * D], F32R, isOutput=False)
    wk = nc.declare_dram_parameter("wk", [L, 128, 5 * D], F32R, isOutput=False)
    wv = nc.declare_dram_parameter("wv", [L, 128, 5 * D], F32R, isOutput=False)
    wo = nc.declare_dram_parameter("wo", [L, 128, 5 * D], F32R, isOutput=False)
    am = nc.declare_dram_parameter("am", [128, 5 * F], F32R, isOutput=False)
    bm = nc.declare_dram_parameter("bm", [F, 5 * 128], F32R, isOutput=False)
    m12 = nc.declare_dram_parameter("m12", [56, 9], F32R, isOutput=False)
    m3 = nc.declare_dram_parameter("m3", [128, 5 * 4], F32R, isOutput=False)
    ffw9 = nc.declare_dram_parameter("ffw9", [9, 13], F32R, isOutput=False)
    ffw3 = nc.declare_dram_parameter("ffw3", [4, 13], F32R, isOutput=False)
    fdw = nc.declare_dram_parameter("fdw", [13, 2], F32R, isOutput=False)
    bo_d = nc.declare_dram_parameter("bo_d", [1, L * 640], F32R, isOutput=False)
    onesw = nc.declare_dram_parameter("onesw", [1, NPC], F32R, isOutput=False)
    y = nc.declare_dram_parameter("y", [NPC, 2], F32, isOutput=True)

    with SplitWaitTileContext(nc) as tc:
        with (
            tc.tile_pool(name="const", bufs=1) as cp,
            tc.tile_pool(name="persist", bufs=1) as pp,
            tc.tile_pool(name="wpool", bufs=1) as wp,
            tc.tile_pool(name="work", bufs=2) as wk_p,
            tc.tile_pool(name="gpool", bufs=2) as gp,
            tc.tile_pool(name="small", bufs=2) as sp,
            tc.tile_pool(name="psA", bufs=2, space="PSUM") as ps_qk,
            tc.tile_pool(name="psB", bufs=1, space="PSUM") as ps_s,
            tc.tile_pool(name="psC", bufs=2, space="PSUM") as ps_vb,
            tc.tile_pool(name="psD", bufs=2, space="PSUM") as ps_o,
            tc.tile_pool(name="psT", bufs=1, space="PSUM") as ps_t,
        ):
            # ---- constants / persistent tiles ----
            idx_sb = cp.tile([128, NT * F], I32)
            nc.sync.dma_start(out=idx_sb[:], in_=idx[:])
            xv_sb = cp.tile([128, NT * F], F32)
            nc.sync.dma_start(out=xv_sb[:], in_=xv[:])
            a_sb = cp.tile([128, 5 * F], F32R)
            nc.sync.dma_start(out=a_sb[:], in_=am[:])
            b_sb = cp.tile([F, 5 * 128], F32R)
            nc.sync.dma_start(out=b_sb[:], in_=bm[:])
            m12_sb = cp.tile([56, 9], F32R)
            nc.sync.dma_start(out=m12_sb[:], in_=m12[:])
            m3_sb = cp.tile([128, 5 * 4], F32R)
            nc.sync.dma_start(out=m3_sb[:], in_=m3[:])
            ffw9_sb = cp.tile([9, 13], F32R)
            nc.sync.dma_start(out=ffw9_sb[:], in_=ffw9[:])
            ffw3_sb = cp.tile([4, 13], F32R)
            nc.sync.dma_start(out=ffw3_sb[:], in_=ffw3[:])
            fdw_sb = cp.tile([13, 2], F32R)
            nc.sync.dma_start(out=fdw_sb[:], in_=fdw[:])
            ident = cp.tile([128, 128], F32)
            make_identity(nc, ident[:])
            bo_sb = cp.tile([1, L * 640], F32R)
            nc.sync.dma_start(out=bo_sb[:], in_=bo_d[:])
            ones_t = cp.tile([1, NB], F32R)
            nc.sync.dma_start(out=ones_t[:], in_=onesw[:, :NB])

            XVT = [pp.tile([128, NPC], F32R, tag=f"xvt{i}", name=f"xvt{i}") for i in range(5)]
            OT = [pp.tile([128, NPC], F32R, tag=f"ot{i}", name=f"ot{i}") for i in range(5)]
            headT = pp.tile([56, NPC], F32R, tag="headT")
            out_sb = pp.tile([128, NT * 2], F32, tag="outsb")

            # ---- phase 0: gather + FM + normalize + transpose, per 128-row tile
            for t in range(NT):
                g = gp.tile([128, F * EW], F32, tag="g")
                for f in range(F):
                    col = t * F + f
                    nc.gpsimd.indirect_dma_start(
                        out=g[:, f * EW : (f + 1) * EW],
                        out_offset=None,
                        in_=tab[:],
                        in_offset=bass.IndirectOffsetOnAxis(
                            ap=idx_sb[:, col : col + 1], axis=0
                        ),
                    )
                g3 = g[:].rearrange("p (f j) -> p f j", j=EW)
                # scale by Xv (also scales the emb1 slot -> f1w)
                nc.vector.tensor_tensor(
                    out=g3,
                    in0=g3,
                    in1=xv_sb[:, t * F : (t + 1) * F]
                    .unsqueeze(2)
                    .to_broadcast([128, F, EW]),
                    op=OP.mult,
                )
                g_xv = g3[:, :, 1:]                      # (p, f, e)
                g_ef = g_xv.transpose([0, 2, 1])         # (p, e, f) view
                xvt_tmp = wk_p.tile([128, D + 1], F32, tag="xvtmp")
                nc.vector.memset(xvt_tmp[:, D : D + 1], 1.0)
                sq_v = xvt_tmp[:, :D].rearrange("p (e f) -> p e f", f=F)
                nc.scalar.activation(out=sq_v, in_=g_ef, func=AF.Square)
                ss = sp.tile([128, 16], F32, tag="ss")
                nc.vector.reduce_sum(out=ss[:], in_=sq_v, axis=mybir.AxisListType.X)
                tt = sp.tile([128, 16], F32, tag="tt")
                nc.vector.reduce_sum(out=tt[:], in_=g_ef, axis=mybir.AxisListType.X)
                mx = sp.tile([128, 16], F32, tag="mx")
                nc.vector.tensor_scalar_max(out=mx[:], in0=ss[:], scalar1=1e-24)
                rt = sp.tile([128, 16], F32, tag="rt")
                nc.scalar.sqrt(out=rt[:], in_=mx[:])
                inv = sp.tile([128, 16], F32, tag="inv")
                nc.vector.reciprocal(out=inv[:], in_=rt[:])
                # normalized xv, contiguous (f, e) layout — reuses sq storage
                xv_v = xvt_tmp[:, :D].rearrange("p (f e) -> p f e", e=E)
                nc.vector.tensor_tensor(
                    out=xv_v,
                    in0=g_xv,
                    in1=inv[:].unsqueeze(1).to_broadcast([128, F, E]),
                    op=OP.mult,
                )
                # head tile: [f1w | f2 | 1]
                head = wk_p.tile([128, 56], F32, tag="head")
                nc.vector.memset(head[:, 55:56], 1.0)
                nc.scalar.activation(out=head[:, 0:F], in_=g3[:, :, 0], func=AF.Copy)
                u = sp.tile([128, 16], F32, tag="u")
                nc.vector.tensor_tensor(out=u[:], in0=tt[:], in1=inv[:], op=OP.mult)
                u2 = sp.tile([128, 16], F32, tag="u2")
                nc.vector.tensor_tensor(out=u2[:], in0=u[:], in1=u[:], op=OP.mult)
                w1 = sp.tile([128, 16], F32, tag="w1")
                nc.vector.tensor_tensor(out=w1[:], in0=ss[:], in1=inv[:], op=OP.mult)
                w2 = sp.tile([128, 16], F32, tag="w2")
                nc.vector.tensor_tensor(out=w2[:], in0=w1[:], in1=inv[:], op=OP.mult)
                dd = sp.tile([128, 16], F32, tag="dd")
                nc.vector.tensor_tensor(
                    out=dd[:], in0=u2[:], in1=w2[:], op=OP.subtract
                )
                nc.vector.tensor_scalar_mul(out=head[:, F:55], in0=dd[:], scalar1=0.5)
                # transposes to feature-major
                for cc in range(5):
                    w = 128 if cc < 4 else 113   # tail chunk carries the ones col
                    pt = ps_t.tile([128, 128], F32, tag="pt")
                    nc.tensor.transpose(
                        out=pt[:w, :],
                        in_=xvt_tmp[:, cc * 128 : cc * 128 + w],
                        identity=ident[:],
                    )
                    nc.vector.tensor_copy(
                        out=XVT[cc][:w, t * 128 : (t + 1) * 128], in_=pt[:w, :]
                    )
                pt = ps_t.tile([128, 128], F32, tag="pt")
                nc.tensor.transpose(out=pt[:56, :], in_=head[:], identity=ident[:])
                nc.vector.tensor_copy(
                    out=headT[:, t * 128 : (t + 1) * 128], in_=pt[:56, :]
                )

            # ---- attention layers: chunk-major so each chunk flows through
            # all 5 layers as soon as its gathers/transposes land; weights are
            # re-streamed per (chunk, layer) to allow it.
            for c in range(NCH):
                for l in range(L):
                    wq_t = wp.tile([128, 5 * D], F32R, tag="wq")
                    nc.sync.dma_start(out=wq_t[:], in_=wq[l, :, :])
                    wk_t = wp.tile([128, 5 * D], F32R, tag="wk")
                    nc.sync.dma_start(out=wk_t[:], in_=wk[l, :, :])
                    wv_t = wp.tile([128, 5 * D], F32R, tag="wv")
                    nc.sync.dma_start(out=wv_t[:], in_=wv[l, :, :])
                    wo_t = wp.tile([128, 5 * D], F32R, tag="wo")
                    nc.sync.dma_start(out=wo_t[:], in_=wo[l, :, :])
                    Xsrc = XVT if l == 0 else OT
                    cs = slice(c * NB, (c + 1) * NB)
                    # scores
                    pss = ps_s.tile([F, NB], F32, tag="s")
                    for m in range(5):
                        mw = MDIMS[m]
                        psq = ps_qk.tile([128, NB], F32, tag="qk")
                        for kb in range(5):
                            kw = KDIMS_AUG[kb]
                            nc.tensor.matmul(
                                out=psq[:mw, :],
                                lhsT=wq_t[:kw, kb * D + m * 128 : kb * D + m * 128 + mw],
                                rhs=XVT[kb][:kw, cs],
                                start=(kb == 0),
                                stop=(kb == 4),
                            )
                        q_sb = wk_p.tile([128, NB], F32, tag="qsb", bufs=1)
                        nc.scalar.activation(
                            out=q_sb[:mw, :], in_=psq[:mw, :], func=AF.Copy
                        )
                        psk = ps_qk.tile([128, NB], F32, tag="qk")
                        for kb in range(5):
                            kw = KDIMS_AUG[kb]
                            nc.tensor.matmul(
                                out=psk[:mw, :],
                                lhsT=wk_t[:kw, kb * D + m * 128 : kb * D + m * 128 + mw],
                                rhs=XVT[kb][:kw, cs],
                                start=(kb == 0),
                                stop=(kb == 4),
                            )
                        p_sb = wk_p.tile([128, NB], F32R, tag="psb", bufs=1)
                        nc.vector.tensor_tensor(
                            out=p_sb[:mw, :],
                            in0=q_sb[:mw, :],
                            in1=psk[:mw, :],
                            op=OP.mult,
                        )
                        nc.tensor.matmul(
                            out=pss[:, :],
                            lhsT=a_sb[:mw, m * F : (m + 1) * F],
                            rhs=p_sb[:mw, :],
                            start=(m == 0),
                            stop=(m == 4),
                        )
                    s_sb = wk_p.tile([F, NB], F32R, tag="ssb", bufs=1)
                    nc.vector.tensor_copy(out=s_sb[:], in_=pss[:])
                    # v, broadcast, att
                    at_tiles = []
                    for m in range(5):
                        mw = MDIMS[m]
                        psv = ps_vb.tile([128, NB], F32, tag="vb")
                        for kb in range(5):
                            kw = KDIMS_AUG[kb]
                            nc.tensor.matmul(
                                out=psv[:mw, :],
                                lhsT=wv_t[:kw, kb * D + m * 128 : kb * D + m * 128 + mw],
                                rhs=Xsrc[kb][:kw, cs],
                                start=(kb == 0),
                                stop=(kb == 4),
                            )
                        v_sb = wk_p.tile([128, NB], F32, tag="vsb", bufs=1)
                        nc.scalar.activation(
                            out=v_sb[:mw, :], in_=psv[:mw, :], func=AF.Copy
                        )
                        psb = ps_vb.tile([128, NB], F32, tag="vb")
                        nc.tensor.matmul(
                            out=psb[:mw, :],
                            lhsT=b_sb[:, m * 128 : m * 128 + mw],
                            rhs=s_sb[:],
                            start=True,
                            stop=True,
                        )
                        at = wk_p.tile([128, NB], F32R, tag=f"at{m}", bufs=1)
                        nc.vector.tensor_tensor(
                            out=at[:mw, :],
                            in0=v_sb[:mw, :],
                            in1=psb[:mw, :],
                            op=OP.mult,
                        )
                        at_tiles.append(at)
                    # out projection (+bo via ones row)
                    for m in range(5):
                        mw = MDIMS[m]
                        ow = 113 if m == 4 else mw   # +1 row: ones for bias path
                        pso = ps_o.tile([128, NB], F32, tag="o")
                        for kb in range(5):
                            kw = KDIMS[kb]
                            nc.tensor.matmul(
                                out=pso[:mw, :],
                                lhsT=wo_t[:kw, kb * D + m * 128 : kb * D + m * 128 + mw],
                                rhs=at_tiles[kb][:kw, :],
                                start=(kb == 0),
                                stop=False,
                            )
                        nc.tensor.matmul(
                            out=pso[:ow, :],
                            lhsT=bo_sb[:, l * 640 + m * 128 : l * 640 + m * 128 + ow],
                            rhs=ones_t[:, :],
                            start=False,
                            stop=True,
                        )
                        nc.vector.tensor_copy(out=OT[m][:ow, cs], in_=pso[:ow, :])

            # ---- head ----
            for c in range(NCH):
                cs = slice(c * NB, (c + 1) * NB)
                ffin12 = wk_p.tile([9, NB], F32R, tag="ffin12")
                ffin3 = wk_p.tile([4, NB], F32R, tag="ffin3")
                ffout = wk_p.tile([13, NB], F32R, tag="ffout")
                ph12 = ps_t.tile([9, NB], F32, tag="pt")
                nc.tensor.matmul(
                    out=ph12[:, :],
                    lhsT=m12_sb[:],
                    rhs=headT[:, cs],
                    start=True,
                    stop=True,
                )
                nc.vector.tensor_copy(out=ffin12[:], in_=ph12[:, :])
                ph3 = ps_t.tile([4, NB], F32, tag="pt")
                for kb in range(5):
                    kw = KDIMS_AUG[kb]
                    nc.tensor.matmul(
                        out=ph3[:, :],
                        lhsT=m3_sb[:kw, kb * 4 : (kb + 1) * 4],
                        rhs=OT[kb][:kw, cs],
                        start=(kb == 0),
                        stop=(kb == 4),
                    )
                nc.vector.tensor_copy(out=ffin3[:], in_=ph3[:, :])
                pf = ps_t.tile([13, NB], F32, tag="pt")
                nc.tensor.matmul(
                    out=pf[:, :], lhsT=ffw9_sb[:], rhs=ffin12[:],
                    start=True, stop=False,
                )
                nc.tensor.matmul(
                    out=pf[:, :], lhsT=ffw3_sb[:], rhs=ffin3[:],
                    start=False, stop=True,
                )
                nc.vector.tensor_scalar_max(out=ffout[:], in0=pf[:, :], scalar1=0.0)
                for q in range(NB // 128):
                    cc = (c * NB) // 128 + q
                    ptot = ps_t.tile([128, 2], F32, tag="pt")
                    nc.tensor.matmul(
                        out=ptot[:, :],
                        lhsT=ffout[:, q * 128 : (q + 1) * 128],
                        rhs=fdw_sb[:],
                        start=True,
                        stop=True,
                    )
                    nc.vector.tensor_copy(
                        out=out_sb[:, cc * 2 : (cc + 1) * 2], in_=ptot[:, :]
                    )

            # final store: out_sb (128, NT, 2) -> y (NT*128, 2)
            nc.sync.dma_start(
                out=y[:].rearrange("(t p) j -> p t j", p=128),
                in_=out_sb[:].rearrange("p (t j) -> p t j", j=2),
            )

    return nc


def host_pack(Xi, Xv, emb1, emb2, Wq, bq, Wk, bk, Wv, bv, Wo, bo,
              m1_w, m1_b, m2_w, m2_b, m3_w, m3_b, ffw_w, ffw_b, fd_w, fd_b):
    """Preprocess full inputs into per-core input maps."""
    idxg = (
        np.arange(F, dtype=np.int64)[None, :] * V + np.asarray(Xi)[:, :, 0]
    ).astype(np.int32)                                    # (N, F)
    Xv = np.asarray(Xv, dtype=np.float32)
    tab = np.concatenate(
        [np.asarray(emb1).reshape(F * V, 1), np.asarray(emb2).reshape(F * V, E)],
        axis=1,
    ).astype(np.float32)                                  # (F*V, 17)

    def pack_w(Wx, bx):
        # (L, D, D)+(L, D) -> (L, 128, 5, D) lhsT tiles, bias at row 112 of kb=4
        out = np.zeros((L, 128, 5, D), dtype=np.float32)
        Wx = np.asarray(Wx, dtype=np.float32)
        for kb in range(5):
            kw = KDIMS[kb]
            out[:, :kw, kb, :] = Wx[:, kb * 128 : kb * 128 + kw, :]
        out[:, 112, 4, :] = np.asarray(bx, dtype=np.float32)
        return round_f32r(out).reshape(L, 128, 5 * D)

    wq_h = pack_w(Wq, bq)
    wk_h = pack_w(Wk, bk)
    wv_h = pack_w(Wv, bv)
    wo_h = pack_w(Wo, bo)

    am_h = np.zeros((128, 5, F), dtype=np.float32)
    for kb in range(5):
        for p in range(KDIMS[kb]):
            d = kb * 128 + p
            am_h[p, kb, d // 16] = 0.25
    am_h = round_f32r(am_h).reshape(128, 5 * F)

    bm_h = np.zeros((F, 5, 128), dtype=np.float32)
    for m in range(5):
        for p in range(MDIMS[m]):
            d = m * 128 + p
            bm_h[d // 16, m, p] = 1.0
    bm_h = round_f32r(bm_h).reshape(F, 5 * 128)

    m12_h = np.zeros((56, 9), dtype=np.float32)
    m12_h[:F, 0:4] = np.asarray(m1_w, dtype=np.float32)
    m12_h[F:55, 4:8] = np.asarray(m2_w, dtype=np.float32)
    m12_h[55, 0:4] = np.asarray(m1_b, dtype=np.float32)
    m12_h[55, 4:8] = np.asarray(m2_b, dtype=np.float32)
    m12_h[55, 8] = 1.0   # ones-row producer (headT row 55 is all-ones)
    m12_h = round_f32r(m12_h)

    m3_h = np.zeros((128, 5, 4), dtype=np.float32)
    for kb in range(5):
        kw = KDIMS[kb]
        m3_h[:kw, kb, :] = np.asarray(m3_w, dtype=np.float32)[
            kb * 128 : kb * 128 + kw, :
        ]
    m3_h[112, 4, :] = np.asarray(m3_b, dtype=np.float32)
    m3_h = round_f32r(m3_h).reshape(128, 5 * 4)

    ffw_w = np.asarray(ffw_w, dtype=np.float32)
    ffw9_h = np.zeros((9, 13), dtype=np.float32)
    ffw9_h[0:8, 0:12] = ffw_w[0:8]
    ffw9_h[8, 0:12] = np.asarray(ffw_b, dtype=np.float32)
    ffw9_h[8, 12] = 1.0   # ones-row producer (ffin12 row 8 is all-ones)
    ffw9_h = round_f32r(ffw9_h)
    ffw3_h = np.zeros((4, 13), dtype=np.float32)
    ffw3_h[:, 0:12] = ffw_w[8:12]
    ffw3_h = round_f32r(ffw3_h)

    fdw_h = np.zeros((13, 2), dtype=np.float32)
    fdw_h[:12] = np.asarray(fd_w, dtype=np.float32)
    fdw_h[12] = np.asarray(fd_b, dtype=np.float32)
    fdw_h = round_f32r(fdw_h)

    bo_h = np.zeros((L, 5, 128), dtype=np.float32)
    bo_a = np.asarray(bo, dtype=np.float32)
    for m in range(5):
        mw = MDIMS[m]
        bo_h[:, m, :mw] = bo_a[:, m * 128 : m * 128 + mw]
    bo_h[:, 4, 112] = 1.0   # ones-row producer for OT[4] bias path
    bo_h = round_f32r(bo_h).reshape(1, L * 640)
    ones_h = np.ones((1, NPC), dtype=np.float32)

    in_maps = []
    for core in range(NCORES):
        sl = slice(core * NPC, (core + 1) * NPC)
        idx_r = (
            idxg[sl].reshape(NT, 128, F).transpose(1, 0, 2).reshape(128, NT * F)
        )
        xv_r = Xv[sl].reshape(NT, 128, F).transpose(1, 0, 2).reshape(128, NT * F)
        in_maps.append(
            dict(
                tab=tab,
                idx=np.ascontiguousarray(idx_r),
                xv=np.ascontiguousarray(xv_r),
                wq=wq_h, wk=wk_h, wv=wv_h, wo=wo_h,
                am=am_h, bm=bm_h, m12=m12_h, m3=m3_h,
                ffw9=ffw9_h, ffw3=ffw3_h, fdw=fdw_h,
                bo_d=bo_h, onesw=ones_h,
            )
        )
    return in_maps


_NC_CACHE = None


def kernel(**inputs):
    global _NC_CACHE, LAST_RESULT
    in_maps = host_pack(**inputs)
    if _NC_CACHE is None:
        _NC_CACHE = build_nc()
    res = run_bass_kernel_spmd(_NC_CACHE, in_maps, list(range(NCORES)))
    LAST_RESULT = res
    out = np.concatenate([res.results[c]["y"] for c in range(NCORES)], axis=0)
    return out


if __name__ == "__main__":
    rng = np.random.default_rng(0)
    print("building...")
    nc = build_nc()
    print("built ok")
